# revision 34
# baseline (speedup 1.0000x reference)
"""Linformer self-attention block on 8 Trainium2 NeuronCores — bf16 v2.

Data-parallel SPMD: the flattened batch b = B*l = 16 is split 2 per core.
Math (per batch, n=4096, c=512, h=8 heads, dh=64, k=256):
    q  = x @ Wq
    xk = proj_k^T @ x            (Linformer folding: proj commutes with Wk)
    xv = proj_v^T @ x
    kp = xk @ Wk ;  vp = xv @ Wv
    dots_h  = (q_h @ kp_h^T) / 8
    attn    = softmax(dots, axis=k)   [no max-subtraction: |dots| < ~6]
    o_h     = attn_h @ vp_h
    y  = o @ Wo + bo

v2 vs v1: all matmuls in bf16 (rel err ~8.5e-3, tol 2e-2), which buys:
  - x is uploaded in BOTH orientations (x and x^T, bf16) so phase B's
    x^T tiles come from plain DMAs — the on-chip PE-transpose is gone.
  - dots runs as per-head K=64 row-tiles at base partitions 0/64: the
    two heads of a pair execute concurrently in disjoint PE row groups
    (f32r needed zero-padded K=128 lhsT, serializing the pair).
  - attn@v and the softmax row-sum matmuls run as M=64 col-tiles at
    out partitions 0/64 — again pairwise-concurrent.
  - HBM traffic roughly halves.
Row sums are materialized pre-broadcast with an all-ones [128,64] lhsT
(engines can't broadcast across partitions); normalization is
reciprocal + scalar_tensor_tensor into the PSUM->SBUF evacuation.
"""
import os
import sys

sys.path.insert(0, "/opt/trn_rl_repo")

KSTAGE = os.environ.get("KSTAGE", "full")  # debug: "a", "full"

import numpy as np
import ml_dtypes
import concourse.bass as bass
import concourse.tile as tile
from concourse import bacc, mybir
from concourse.bass_utils import run_bass_kernel_spmd

F32 = mybir.dt.float32
F32R = mybir.dt.float32r
BF16 = mybir.dt.bfloat16
AF = mybir.ActivationFunctionType
OP = mybir.AluOpType

B, L, SEQ, DIM = 2, 8, 4096, 512
H, DH, KL = 8, 64, 256
NCORES = 8
BPC = (B * L) // NCORES   # batches per core
NT = SEQ // 128           # 32 row-tiles
NCH = SEQ // 512          # 8 row-chunks
SCALE = float(DH) ** -0.5
NPBF = ml_dtypes.bfloat16
XTOFF = SEQ * DIM            # xt offset inside xall's per-batch flat dim
PKVOFF, WOFF, BOOFF = 0, SEQ * DIM, SEQ * DIM + DIM * 4 * DIM
WBLEN = BOOFF + DIM


def _phase_a(tc, ps, sb, xall, bi, pkv_sb, w_sb, kpt_sb, vp_sb):
    """Per-batch Linformer fold: xkvT = x^T @ [proj_k|proj_v], then
    kpT = Wk^T @ xkT (kept [d-pair, kt2, k]) and vp = xvT^T @ Wv
    (kept [k, kt2, d])."""
    nc = tc.nc
    # x tiles for this batch: 8 chunks of [128, 4, 512] bf16 (1 MB DMAs),
    # all resident until the 4 ct-passes below are done.
    xa = [sb.tile([128, 4, DIM], BF16, tag=f"xa{i}", name=f"xa{bi}_{i}")
          for i in range(8)]
    for i in range(8):
        nc.sync.dma_start(
            xa[i][:],
            xall[bi, i * 262144:(i + 1) * 262144].rearrange(
                "(ntl p c) -> p ntl c", p=128, c=DIM))

    # xkvT[c, kk] = sum_n x[n, c] * pkv[n, kk]; one ct-slice (128 c rows)
    # per PSUM bank pass.
    xkv_sb = sb.tile([128, 4, DIM], BF16, tag="xkv", bufs=2)
    for ct in range(4):
        xkv_ps = ps.tile([128, DIM], F32, tag="dots", bufs=4)
        for i in range(8):
            for j in range(4):
                nt = i * 4 + j
                nc.tensor.matmul(
                    xkv_ps[:],
                    xa[i][:, j, ct * 128:(ct + 1) * 128],
                    pkv_sb[nt // 8][:, nt % 8, :],
                    start=(i == 0 and j == 0), stop=(i == 7 and j == 3))
        nc.vector.tensor_copy(xkv_sb[:, ct, :], xkv_ps[:])

    # kpT[d, k] = sum_c Wk[c, d] * xkT[c, k]; dt == head-pair hp.
    for hp in range(4):
        kpt_ps = ps.tile([128, KL], F32, tag="dots", bufs=4)
        for cc in range(4):
            nc.tensor.matmul(
                kpt_ps[:],
                w_sb[:, cc, DIM + hp * 128:DIM + (hp + 1) * 128],
                xkv_sb[:, cc, 0:KL],
                start=(cc == 0), stop=(cc == 3))
        nc.vector.tensor_copy(kpt_sb[:, hp, :], kpt_ps[:])

    # vp[k, d] = sum_c xvT[c, k] * Wv[c, d]
    for kt2 in range(2):
        vp_ps = ps.tile([128, DIM], F32, tag="dots", bufs=4)
        for cc in range(4):
            nc.tensor.matmul(
                vp_ps[:],
                xkv_sb[:, cc, KL + kt2 * 128:KL + (kt2 + 1) * 128],
                w_sb[:, cc, 2 * DIM:3 * DIM],
                start=(cc == 0), stop=(cc == 3))
        nc.vector.tensor_copy(vp_sb[:, kt2, :], vp_ps[:])


def _phase_b(tc, ps, sb, xall, y, bi, consts, kpt_sb, vp_sb):
    nc = tc.nc
    w_sb, ones_bf, bo_bcast = consts
    xt_full = xall[bi, XTOFF:XTOFF + SEQ * DIM].rearrange(
        "(cc p n) -> p cc n", p=128, n=SEQ)
    for nj in range(NCH):
        ns = slice(nj * 512, (nj + 1) * 512)
        # x^T chunk [128, 4(cc), 512] straight from the transposed upload.
        xt_t = sb.tile([128, 4, 512], BF16, tag="xt", bufs=3)
        nc.sync.dma_start(xt_t[:], xt_full[:, :, ns])

        # qT[d, n] = sum_c Wq[c, d] * xT[c, n]; dt == head-pair hp.
        qt_sb = []
        for hp in range(4):
            qt_ps = ps.tile([128, 512], F32, tag="qt", bufs=2)
            for cc in range(4):
                nc.tensor.matmul(
                    qt_ps[:],
                    w_sb[:, cc, hp * 128:(hp + 1) * 128],
                    xt_t[:, cc, :],
                    start=(cc == 0), stop=(cc == 3))
            qt = sb.tile([128, 512], BF16, tag="qtsb", bufs=8,
                         name=f"qt{bi}_{nj}_{hp}")
            nc.vector.tensor_copy(qt[:], qt_ps[:])
            qt_sb.append(qt)

        ot_sb = []
        for hp in range(4):
            # dotsT_h[k, n] = sum_dh kpT_h[dh, k] * qT_h[dh, n]
            # Per-head K=64 row-tiles at partitions 0/64. kt2-outer issue
            # order keeps consecutive matmuls in DISJOINT PE row groups so
            # the head pair executes concurrently (strict-FIFO issue).
            exp_tiles = [
                sb.tile([128, 2, 512], BF16, tag="exp", bufs=4,
                        name=f"exp{bi}_{nj}_{2*hp+hi}")
                for hi in range(2)]
            for kt2 in range(2):
                for hi in range(2):
                    rs = slice(hi * 64, hi * 64 + 64)
                    dots_ps = ps.tile([128, 512], F32, tag="dots", bufs=4)
                    nc.tensor.matmul(
                        dots_ps[:],
                        kpt_sb[rs, hp, kt2 * 128:(kt2 + 1) * 128],
                        qt_sb[hp][rs, :],
                        start=True, stop=True)
                    nc.scalar.activation(
                        exp_tiles[hi][:, kt2, :], dots_ps[:], AF.Exp,
                        scale=SCALE)

            # o pair tile: head hi's M=64 col-tile lands at out partitions
            # hi*64 (pairwise-concurrent col groups); row sums via all-ones
            # lhsT to the same split.
            # den borrows a dots slot (all four are drained by exp before
            # the row sums issue); the freed bank double-buffers os so the
            # next pair's attn@v doesn't wait on this pair's normalize.
            os_ps = ps.tile([128, 512], F32, tag="os", bufs=2)
            den_ps = ps.tile([128, 512], F32, tag="dots", bufs=4)
            for kt2 in range(2):
                for hi in range(2):
                    rs = slice(hi * 64, hi * 64 + 64)
                    nc.tensor.matmul(
                        os_ps[rs, :],
                        vp_sb[:, kt2, hp * 128 + hi * 64:
                              hp * 128 + hi * 64 + 64],
                        exp_tiles[hi][:, kt2, :],
                        start=(kt2 == 0), stop=(kt2 == 1))
            # Row sums: col-tiled ones-matmuls, pairwise-concurrent.
            # (A DVE pre-add of the k-halves would halve the PE cost here
            # but pushes DVE busy past PE — measured worse.)
            for kt2 in range(2):
                for hi in range(2):
                    rs = slice(hi * 64, hi * 64 + 64)
                    nc.tensor.matmul(
                        den_ps[rs, :],
                        ones_bf[:],
                        exp_tiles[hi][:, kt2, :],
                        start=(kt2 == 0), stop=(kt2 == 1))
            rec = sb.tile([128, 512], F32, tag="rec", bufs=2)
            nc.vector.reciprocal(rec[:], den_ps[:])
            ot = sb.tile([128, 512], BF16, tag="ot", bufs=8,
                         name=f"ot{bi}_{nj}_{hp}")
            nc.vector.scalar_tensor_tensor(
                ot[:], os_ps[:], 1.0, rec[:], op0=OP.mult, op1=OP.mult)
            ot_sb.append(ot)

        # y[n, d] = sum_do oT[do, n] * Wo[do, d] + bo   (bf16 store halves
        # the output write traffic and the per-call result buffers)
        yo_sb = sb.tile([128, 4, DIM], BF16, tag="yo", bufs=2)
        for ntl in range(4):
            y_ps = ps.tile([128, 512], F32, tag="qt", bufs=2)
            for hp in range(4):
                nc.tensor.matmul(
                    y_ps[:],
                    ot_sb[hp][:, ntl * 128:(ntl + 1) * 128],
                    w_sb[:, hp, 3 * DIM:4 * DIM],
                    start=(hp == 0), stop=(hp == 3))
            nc.vector.scalar_tensor_tensor(
                yo_sb[:, ntl, :], y_ps[:], 1.0, bo_bcast[:],
                op0=OP.mult, op1=OP.add)
        nc.sync.dma_start(
            y[bi, ns, :].rearrange("(ntl p) c -> p ntl c", p=128),
            yo_sb[:])


def _body(tc, ctx, xall, wb, y):
    nc = tc.nc
    const = ctx.enter_context(tc.tile_pool(name="const", bufs=1))
    sb = ctx.enter_context(tc.tile_pool(name="sb", bufs=1))
    ps = ctx.enter_context(tc.tile_pool(name="ps", bufs=1, space="PSUM"))

    # ---- resident constants (wb packs [pkv | Wq|Wk|Wv|Wo | bo] flat) ----
    w_sb = const.tile([128, 4, 4 * DIM], BF16)
    nc.sync.dma_start(
        w_sb[:],
        wb[0, WOFF:WOFF + DIM * 4 * DIM].rearrange(
            "(cc p d) -> p cc d", p=128, d=4 * DIM))
    pkv_t = [const.tile([128, 8, DIM], BF16, name=f"pkv{i}")
             for i in range(4)]
    for i in range(4):
        nc.sync.dma_start(
            pkv_t[i][:],
            wb[0, i * 8 * 128 * DIM:(i + 1) * 8 * 128 * DIM].rearrange(
                "(nt p k) -> p nt k", p=128, k=DIM))

    ones_st = const.tile([128, 128], F32)
    nc.vector.memset(ones_st[:], 1.0)
    ones_bf = const.tile([128, 64], BF16)
    nc.vector.tensor_copy(ones_bf[:], ones_st[:, 0:64])
    ones1 = const.tile([1, 128], BF16)
    nc.vector.tensor_copy(ones1[:], ones_st[0:1, :])

    bo_row = const.tile([1, DIM], BF16)
    nc.sync.dma_start(bo_row[:], wb[0:1, BOOFF:BOOFF + DIM])
    bo_bcast = const.tile([128, DIM], F32)
    bo_ps = ps.tile([128, DIM], F32, tag="dots", bufs=4)
    nc.tensor.matmul(bo_ps[:], ones1[:], bo_row[:], start=True, stop=True)
    nc.vector.tensor_copy(bo_bcast[:], bo_ps[:])

    # per-batch Linformer products, alive from phase A to end of phase B
    kpt_sb = [const.tile([128, 4, KL], BF16, name=f"kpt{i}")
              for i in range(BPC)]
    vp_sb = [const.tile([128, 2, DIM], BF16, name=f"vp{i}")
             for i in range(BPC)]

    consts = (w_sb, ones_bf, bo_bcast)
    # KREPEAT>1 re-runs the whole computation in one NEFF; profiling-only
    # knob (T(r2)-T(r1) cancels the per-launch dispatch overhead).
    for _rep in range(int(os.environ.get("KREPEAT", "1"))):
        for bi in range(BPC):
            _phase_a(tc, ps, sb, xall, bi, pkv_t, w_sb, kpt_sb[bi],
                     vp_sb[bi])
            if KSTAGE != "a":
                _phase_b(tc, ps, sb, xall, y, bi, consts, kpt_sb[bi],
                         vp_sb[bi])


def _build():
    from contextlib import ExitStack
    nc = bacc.Bacc("TRN2", target_bir_lowering=False, debug=False,
                   num_devices=NCORES)
    xall = nc.declare_dram_parameter("xall", [BPC, 2 * SEQ * DIM], BF16,
                                     isOutput=False)
    wb = nc.declare_dram_parameter("wb", [1, WBLEN], BF16, isOutput=False)
    y = nc.declare_dram_parameter("y", [BPC, SEQ, DIM], BF16, isOutput=True)
    with tile.TileContext(nc) as tc, ExitStack() as ctx:
        _body(tc, ctx, xall, wb, y)
    nc.compile()
    return nc


_prog = None


def _get_prog():
    global _prog
    if _prog is None:
        _prog = _build()
    return _prog


def make_per_core_inputs(inputs):
    """Host-side prep shared by kernel() and benches: bf16 casts, the
    transposed-x upload, and the packed [pkv | W | bo] blob."""
    x32 = np.asarray(inputs["x"], dtype=np.float32).reshape(B * L, SEQ, DIM)
    xb = np.ascontiguousarray(x32).astype(NPBF).reshape(B * L, SEQ * DIM)
    xtb = np.ascontiguousarray(x32.transpose(0, 2, 1)).astype(NPBF).reshape(
        B * L, SEQ * DIM)
    xall = np.concatenate([xb, xtb], axis=1)
    w = np.concatenate(
        [np.asarray(inputs[k], dtype=np.float32) for k in
         ("Wq", "Wk", "Wv", "Wo")], axis=1).astype(NPBF)
    pkv = np.concatenate(
        [np.asarray(inputs["proj_k"], dtype=np.float32),
         np.asarray(inputs["proj_v"], dtype=np.float32)], axis=1).astype(NPBF)
    bo = np.asarray(inputs["bo"], dtype=np.float32).astype(NPBF)
    wb = np.concatenate(
        [pkv.reshape(-1), w.reshape(-1), bo.reshape(-1)]).reshape(1, WBLEN)
    return {"xall": np.ascontiguousarray(xall),
            "wb": np.ascontiguousarray(wb)}


def kernel(x, Wq, Wk, Wv, proj_k, proj_v, Wo, bo, _trace=False):
    pc = make_per_core_inputs(dict(
        x=x, Wq=Wq, Wk=Wk, Wv=Wv, proj_k=proj_k, proj_v=proj_v, Wo=Wo,
        bo=bo))
    in_maps = [
        {"xall": pc["xall"][c * BPC:(c + 1) * BPC], "wb": pc["wb"]}
        for c in range(NCORES)
    ]
    res = run_bass_kernel_spmd(
        _get_prog(), in_maps, core_ids=list(range(NCORES)), trace=_trace)
    out = np.concatenate(
        [np.asarray(res.results[c]["y"], dtype=np.float32)
         for c in range(NCORES)], axis=0)
    if _trace:
        kernel._last = res
    return out.reshape(B, L, SEQ, DIM)
